# revision 13
# baseline (speedup 1.0000x reference)
"""Trainium2 Bass kernel for nn_ConcatHeadModule (pairwise MLP scores).

scores[i, j] = W_out . tanh(th[i] + tm[j] + hid2_bias) + out_bias
  th = tanh(xf @ W_foh + cat_bias[:H]) @ W_hid2[:H]
  tm = tanh(xf @ W_fom + cat_bias[H:]) @ W_hid2[H:]

tanh(a+b) is replaced by a low-rank separable expansion fitted on the
empirical (a, b) value distribution (host-side LS fit on quantile
grids):

  tanh(a+b) ~= sum_{q<Q} sum_{f<NB} T_q(a/ascale) * G[q,f] * g_f(b)

with T_q Chebyshev polynomials (stride-2 DVE recurrence, 128 wide) and
g_f in {1, b, tanh(b + s_k)} (+ optionally {b^2, b^3}).  The pairwise
scores then become 4 accumulating bf16 matmuls per output column chunk
with contraction dim NB*D = 512.

Layout/schedule (v2):
 - B side is COLUMN-CHUNKED: x^T arrives as 4 column-quarter images on
   3 DMA queues; each quarter flows tm -> tanhm -> ptm(half) ->
   B-feature ACTs(half) -> final matmuls(quarter) -> evac -> store,
   so output DMA overlaps compute.
 - th/tm projections use duplicated stationaries [w|w] so PSUM holds
   results on BOTH partition halves; feature ACTs run full-width
   straight from PSUM.
 - Mixing runs s-outer in two passes over two PSUM banks so it
   pipelines behind the Chebyshev recurrence.
 - Final contraction is bf16 (At/Bt bf16), PSUM f32.
 - All engines are used: PE (matmuls), Scalar (tanh ACTs), DVE
   (recurrence + casts + evac), GpSimd (copies + evac + DMA), Sync/
   Scalar/GpSimd DMA queues balanced by arrival deadlines.

Sharding: rows i split across 8 cores (128 rows each); everything else
replicated.
"""

import sys

sys.path.insert(0, "/opt/trn_rl_repo")

import numpy as np

import concourse.bass as bass
from concourse.alu_op_type import AluOpType
import concourse.tile as tile
from concourse import bacc, mybir
from concourse.bass_utils import run_bass_kernel_spmd

N = 1024          # nodes
F = 512           # 2 * LDIMS
H = 128           # hidden
D = 64            # hid2
NCORES = 8
R = N // NCORES   # rows per core = 128

Q = 8             # Chebyshev degree count (a-side)
NS = Q // 2       # Chebyshev pair tiles = 4
NB = 8            # b-side features
NCH = NB // 2     # 128-partition chunks in the final contraction = 4
NPOLY = 0         # 0 (safe: 6 tanh feats) or 2 (mixed: b^2,b^3 + 4 tanh)
NTANH = NB - 2 - NPOLY
NTP = NTANH // 2  # tanh feature pairs on scalar engine

CLEN = 256        # output column chunk
NCHK = N // CLEN  # 4
HLEN = 512        # half width (ptm / B-ACT granularity)

F32 = mybir.dt.float32
BF16 = mybir.dt.bfloat16
Tanh = mybir.ActivationFunctionType.Tanh
Copy = mybir.ActivationFunctionType.Copy


def _cheb(x, n):
    T = np.empty(x.shape + (n,))
    T[..., 0] = 1.0
    T[..., 1] = x
    for q in range(2, n):
        T[..., q] = 2 * x * T[..., q - 1] - T[..., q - 2]
    return T


def _build_program(out_bias: float = 0.0):
    # out_bias is folded into the smix input data; the program itself is
    # independent of it (arg kept for test-harness compatibility).
    nc = bacc.Bacc("TRN2", target_bir_lowering=False, debug=False,
                   num_devices=NCORES)

    # DRAM inputs.  aw = [wfohp (4H) | wh2dup (4D)] on the gpsimd queue.
    # smix is built ON DEVICE (it is 1.6% dense block-diagonal): a [H, D]
    # 0/1 diagonal mask is scaled per-partition by value vectors shipped
    # in the bias image (cols 8..8+2*NS*NCH).
    xtq_d = [nc.dram_tensor(f"xtq{c}", [H, 4 * CLEN], BF16,
                            kind="ExternalInput") for c in range(NCHK)]
    xtmp_d = nc.dram_tensor("xtmp", [H, 4 * R], BF16, kind="ExternalInput")
    aw_d = nc.dram_tensor("aw", [H, 4 * H + 4 * D], BF16,
                          kind="ExternalInput")
    wfomp_d = nc.dram_tensor("wfomp", [H, 4 * H], BF16, kind="ExternalInput")
    mask_d = nc.dram_tensor("mask", [H, D], BF16, kind="ExternalInput")
    bias_d = nc.dram_tensor("bias", [H, 8 + 2 * NS * NCH], F32,
                            kind="ExternalInput")
    out_d = nc.dram_tensor("out", [R, N], F32, kind="ExternalOutput")

    with tile.TileContext(nc) as tc:
        with (
            tc.tile_pool(name="consts", bufs=1) as consts,
            tc.tile_pool(name="work", bufs=1) as work,
            tc.tile_pool(name="scr", bufs=2) as scrp,
            tc.tile_pool(name="stage", bufs=1) as stagep,
            tc.tile_pool(name="ps", bufs=2, space="PSUM") as psp,
            tc.tile_pool(name="ptm", bufs=2, space="PSUM") as ptmp,
            tc.tile_pool(name="ptile", bufs=2, space="PSUM") as tmp_,
            tc.tile_pool(name="pmix", bufs=2, space="PSUM") as pmixp,
        ):
            # Trigger the tanh ACT table load immediately.
            warm = consts.tile([H, 1], F32, tag="warm")
            nc.vector.memset(warm[:], 0.0)

            # ---- input DMAs, balanced by arrival deadline ----
            xtq = [consts.tile([H, 4 * CLEN], BF16, tag=f"xtq{c}",
                               name=f"xtq{c}") for c in range(NCHK)]
            xtmp = consts.tile([H, 4 * R], BF16, tag="xtmp")
            aw = consts.tile([H, 4 * H + 4 * D], BF16, tag="aw")
            wfomp = consts.tile([H, 4 * H], BF16, tag="wfomp")
            mask = consts.tile([H, D], BF16, tag="mask")
            smix = work.tile([H, NS * NCH * H], BF16, tag="smix")
            biases = consts.tile([H, 8 + 2 * NS * NCH], F32, tag="biases")

            # sync queue: xtmp, bias, xtq1, xtq3 (+ outputs 0,1 later)
            nc.sync.dma_start(xtmp[:], xtmp_d[:])
            nc.sync.dma_start(biases[:], bias_d[:])
            nc.sync.dma_start(xtq[1][:], xtq_d[1][:])
            nc.sync.dma_start(xtq[3][:], xtq_d[3][:])
            # scalar queue: wfomp, xtq0 (engine then does ACT chain)
            nc.scalar.dma_start(wfomp[:], wfomp_d[:])
            nc.scalar.dma_start(xtq[0][:], xtq_d[0][:])
            # gpsimd queue: mask, aw, xtq2 (+ outputs 2,3 later)
            nc.gpsimd.dma_start(mask[:], mask_d[:])
            nc.gpsimd.dma_start(aw[:], aw_d[:])
            nc.gpsimd.dma_start(xtq[2][:], xtq_d[2][:])

            nc.scalar.activation(warm[:], warm[:], Tanh)

            # ---- build smix on GpSimd: 32 x [H, D] per-partition scaled
            # copies of the diagonal mask (s-major so pass A unblocks first)
            for s in range(NS):
                for c in range(NCH):
                    blk = (s * NCH + c) * H
                    for fl in range(2):
                        vcol = 8 + 2 * (s * NCH + c) + fl
                        nc.gpsimd.tensor_scalar_mul(
                            smix[:, blk + fl * D:blk + (fl + 1) * D],
                            mask[:], biases[:, vcol:vcol + 1])

            wfohp = aw[:, 0:4 * H]
            wh2t = aw[:, 4 * H:4 * H + 2 * D]        # [wh2t|wh2t]
            wh2b = aw[:, 4 * H + 2 * D:4 * H + 4 * D]  # [wh2b|wh2b]
            cbm = biases[:, 0:1]
            cbh = biases[:, 1:2]
            rascale = biases[:, 5:6]

            # B-feature tiles (bf16), full width, written per chunk.
            Bt = [work.tile([2 * D, N], BF16, tag=f"B{c}", name=f"B{c}")
                  for c in range(NCH)]
            # ones feature: top half of Bt[0]
            nc.vector.memset(Bt[0][0:D, :], 1.0)

            # ---- A side: proj-h -> tanhh -> th ----
            pm2 = psp.tile([H, CLEN], F32, tag="ps", name="pm2")
            for q in range(4):
                nc.tensor.matmul(pm2[:, 0:R], wfohp[:, q * H:(q + 1) * H],
                                 xtmp[:, q * R:(q + 1) * R],
                                 start=(q == 0), stop=(q == 3))
            tanhh = work.tile([H, R], BF16, tag="tanhh")
            nc.scalar.activation(tanhh[:], pm2[:, 0:R], Tanh, bias=cbh)
            ps3 = psp.tile([H, CLEN], F32, tag="ps", name="ps3")
            nc.tensor.matmul(ps3[:, 0:R], wh2t, tanhh[:],
                             start=True, stop=True)

            # ---- B side per-chunk: tm -> tanhm ----
            tanhm = [work.tile([H, CLEN], BF16, tag=f"tanhm{c}",
                               name=f"tanhm{c}") for c in range(NCHK)]

            def tm_chunk(c):
                pm = tmp_.tile([H, CLEN], F32, tag="tm", name=f"tm{c}")
                for q in range(4):
                    nc.tensor.matmul(pm[:], wfomp[:, q * H:(q + 1) * H],
                                     xtq[c][:, q * CLEN:(q + 1) * CLEN],
                                     start=(q == 0), stop=(q == 3))
                nc.scalar.activation(tanhm[c][:], pm[:], Tanh, bias=cbm)

            # ---- A-side Chebyshev chain on DVE ----
            def a_chain():
                arep = work.tile([2 * D, R], F32, tag="arep")
                nc.vector.tensor_scalar_mul(arep[:], ps3[:, 0:R], rascale)
                sq = work.tile([2 * D, R], F32, tag="sq")
                nc.vector.tensor_mul(sq[:], arep[:], arep[:])
                M2 = work.tile([2 * D, R], F32, tag="M2")
                nc.vector.tensor_scalar(M2[:], sq[:], 4.0, -2.0,
                                        AluOpType.mult, AluOpType.add)
                Pf = [work.tile([2 * D, R], F32, tag=f"Pf{s}", name=f"Pf{s}")
                      for s in range(NS)]
                Pb = [work.tile([2 * D, R], BF16, tag=f"Pb{s}",
                                name=f"Pb{s}") for s in range(NS)]
                nc.vector.memset(Pf[0][0:D, :], 1.0)
                nc.vector.tensor_copy(Pf[0][D:2 * D, :], arep[D:2 * D, :])
                nc.vector.tensor_copy(Pb[0][:], Pf[0][:])
                nc.vector.tensor_scalar(Pf[1][0:D, :], sq[0:D, :], 2.0, -1.0,
                                        AluOpType.mult, AluOpType.add)
                scr0 = scrp.tile([2 * D, R], F32, tag="scr")
                nc.vector.tensor_mul(scr0[D:2 * D, :], M2[D:2 * D, :],
                                     arep[D:2 * D, :])
                nc.vector.tensor_sub(Pf[1][D:2 * D, :], scr0[D:2 * D, :],
                                     arep[D:2 * D, :])
                nc.vector.tensor_copy(Pb[1][:], Pf[1][:])
                for s in range(2, NS):
                    scr = scrp.tile([2 * D, R], F32, tag="scr2",
                                    name=f"scr{s}")
                    nc.vector.tensor_mul(scr[:], M2[:], Pf[s - 1][:])
                    nc.vector.tensor_sub(Pf[s][:], scr[:], Pf[s - 2][:])
                    nc.vector.tensor_copy(Pb[s][:], Pf[s][:])
                return Pb

            # ---- schedule ----
            tm_chunk(0)
            a_chain_Pb = a_chain()
            tm_chunk(1)

            # ptm half 0
            ptm0 = ptmp.tile([2 * D, HLEN], F32, tag="ptm", name="ptm0")
            nc.tensor.matmul(ptm0[:, 0:CLEN], wh2b, tanhm[0][:],
                             start=True, stop=True, skip_group_check=True)
            nc.tensor.matmul(ptm0[:, CLEN:], wh2b, tanhm[1][:],
                             start=True, stop=True, skip_group_check=True)

            # B features half 0 (scalar tanh pairs; gpsimd raw-b copy)
            for k in range(NTP):
                nc.scalar.activation(Bt[1 + NPOLY // 2 + k][:, 0:HLEN],
                                     ptm0[:], Tanh,
                                     bias=biases[:, 2 + k:3 + k])
            nc.vector.tensor_copy(Bt[0][D:2 * D, 0:HLEN], ptm0[D:2 * D, :])
            if NPOLY:
                h2bv = biases[:, 6:7]
                bb0 = scrp.tile([2 * D, HLEN], F32, tag="bb", name="bb0")
                nc.vector.tensor_scalar(bb0[:], ptm0[:], h2bv, 0.0,
                                        AluOpType.add, AluOpType.bypass)
                zz0 = scrp.tile([2 * D, HLEN], F32, tag="zz", name="zz0")
                nc.vector.tensor_mul(zz0[:], bb0[:], bb0[:])
                nc.vector.tensor_copy(Bt[1][0:D, 0:HLEN], zz0[0:D, :])
                nc.vector.tensor_mul(Bt[1][D:2 * D, 0:HLEN],
                                     zz0[D:2 * D, :], bb0[D:2 * D, :])

            # mixing pass A (chunks 0,1), s-outer
            pA = [pmixp.tile([H, R], F32, tag="pmix", name=f"pA{c}")
                  for c in range(NCH)]
            At = [work.tile([H, R], BF16, tag=f"A{c}", name=f"A{c}")
                  for c in range(NCH)]
            tm_chunk(2)
            for s in range(NS):
                for c in (0, 1):
                    blk = (s * NCH + c) * H
                    nc.tensor.matmul(pA[c][:], smix[:, blk:blk + H],
                                     a_chain_Pb[s][:],
                                     start=(s == 0), stop=(s == NS - 1),
                                     skip_group_check=True)
            nc.vector.tensor_copy(At[0][:], pA[0][:])
            nc.vector.tensor_copy(At[1][:], pA[1][:])

            tm_chunk(3)
            # ptm half 1
            ptm1 = ptmp.tile([2 * D, HLEN], F32, tag="ptm", name="ptm1")
            nc.tensor.matmul(ptm1[:, 0:CLEN], wh2b, tanhm[2][:],
                             start=True, stop=True, skip_group_check=True)
            nc.tensor.matmul(ptm1[:, CLEN:], wh2b, tanhm[3][:],
                             start=True, stop=True, skip_group_check=True)

            # mixing pass B (chunks 2,3)
            for s in range(NS):
                for c in (2, 3):
                    blk = (s * NCH + c) * H
                    nc.tensor.matmul(pA[c][:], smix[:, blk:blk + H],
                                     a_chain_Pb[s][:],
                                     start=(s == 0), stop=(s == NS - 1),
                                     skip_group_check=True)
            nc.vector.tensor_copy(At[2][:], pA[2][:])
            nc.vector.tensor_copy(At[3][:], pA[3][:])

            # B features half 1
            for k in range(NTP):
                nc.scalar.activation(Bt[1 + NPOLY // 2 + k][:, HLEN:],
                                     ptm1[:], Tanh,
                                     bias=biases[:, 2 + k:3 + k])
            nc.vector.tensor_copy(Bt[0][D:2 * D, HLEN:], ptm1[D:2 * D, :])
            if NPOLY:
                h2bv = biases[:, 6:7]
                bb1 = scrp.tile([2 * D, HLEN], F32, tag="bb", name="bb1")
                nc.vector.tensor_scalar(bb1[:], ptm1[:], h2bv, 0.0,
                                        AluOpType.add, AluOpType.bypass)
                zz1 = scrp.tile([2 * D, HLEN], F32, tag="zz", name="zz1")
                nc.gpsimd.tensor_mul(zz1[:], bb1[:], bb1[:])
                nc.gpsimd.tensor_copy(Bt[1][0:D, HLEN:], zz1[0:D, :])
                nc.gpsimd.tensor_mul(Bt[1][D:2 * D, HLEN:],
                                     zz1[D:2 * D, :], bb1[D:2 * D, :])

            # ---- final contraction + store, per column chunk ----
            evac_engine = [nc.vector, nc.vector, nc.scalar, nc.scalar]
            dma_engine = [nc.sync, nc.sync, nc.gpsimd, nc.gpsimd]
            for ck in range(NCHK):
                mv = slice(ck * CLEN, (ck + 1) * CLEN)
                psc = psp.tile([H, CLEN], F32, tag="ps", name=f"psc{ck}")
                corder = (1, 2, 3, 0)   # raw-b feature pair arrives last
                for oi, ci in enumerate(corder):
                    nc.tensor.matmul(psc[:], At[ci][:], Bt[ci][:, mv],
                                     start=(oi == 0), stop=(oi == NCH - 1),
                                     skip_group_check=True)
                stg = stagep.tile([R, CLEN], F32, tag=f"stg{ck}",
                                  name=f"stg{ck}")
                if evac_engine[ck] is nc.scalar:
                    nc.scalar.activation(stg[:], psc[:], Copy)
                else:
                    evac_engine[ck].tensor_copy(stg[:], psc[:])
                dma_engine[ck].dma_start(out_d[:, mv], stg[:])

    nc.compile()
    return nc


def _fit_G(a_samp, b_samp, ascale, bsh):
    """LS fit of tanh(a+b) on empirical quantile grids."""
    na = 301
    qs = np.linspace(0, 1, na)
    ag = np.quantile(a_samp, qs)
    ag = np.concatenate([ag, np.linspace(ag[0] * 1.08, ag[-1] * 1.08, 32)])
    bg = np.quantile(b_samp, qs)
    bg = np.concatenate([bg, np.linspace(bg[0] * 1.08, bg[-1] * 1.08, 32)])
    M = np.tanh(ag[:, None] + bg[None, :])
    Fa = _cheb(np.clip(ag / ascale, -1, 1), Q)
    feats = [np.ones_like(bg), bg]
    for i in range(NPOLY):
        feats.append(bg ** (2 + i))
    for c in bsh:
        feats.append(np.tanh(bg + c))
    Fb = np.stack(feats, 1)
    lam = 1e-7
    G = np.linalg.solve(Fa.T @ Fa + lam * np.eye(Q), Fa.T @ M @ Fb)
    G = G @ np.linalg.inv(Fb.T @ Fb + lam * np.eye(NB))
    return G


def _make_in_maps(x, W_foh, W_fom, cat_bias, W_hid2, hid2_bias, W_out,
                  out_bias=0.0):
    import ml_dtypes

    def tobf(a):
        return np.asarray(a, np.float32).astype(ml_dtypes.bfloat16)

    def bfval(a):
        return np.asarray(a, np.float32).astype(
            ml_dtypes.bfloat16).astype(np.float32)

    xf = x.reshape(N, F)
    xt = np.ascontiguousarray(xf.T)                      # [F, N]

    # p-major packing: img[p, q*C + j] = src[q*128 + p, j]
    def pack(src):
        C = src.shape[1]
        return np.ascontiguousarray(
            src.reshape(4, H, C).transpose(1, 0, 2).reshape(H, 4 * C))

    xtp = pack(xt)                                       # [H, 4*N]
    # column chunks: [H, 4*CLEN] each
    xtq = [tobf(np.ascontiguousarray(
        xtp.reshape(H, 4, N)[:, :, c * CLEN:(c + 1) * CLEN]
        .reshape(H, 4 * CLEN))) for c in range(NCHK)]
    wfomp = tobf(pack(W_fom))
    wfohp = pack(W_foh)
    wh2dup = np.concatenate([W_hid2[:H], W_hid2[:H],
                             W_hid2[H:], W_hid2[H:]], axis=1)
    aw = tobf(np.concatenate([wfohp, wh2dup], axis=1))

    # --- empirical a/b samples (match device numerics: bf16 inputs) ---
    w = W_out[:, 0].astype(np.float64)
    h2b = hid2_bias.astype(np.float64)
    xq = bfval(xf)
    headfov = xq @ bfval(W_foh)
    modfov = xq @ bfval(W_fom)
    tanhh = bfval(np.tanh(headfov + cat_bias[:H]))
    tanhm = bfval(np.tanh(modfov + cat_bias[H:]))
    wh2q = bfval(W_hid2)
    a = tanhh @ wh2q[:H]
    b = tanhm @ wh2q[H:] + h2b
    ascale = float(np.abs(a).max()) * 1.02
    bsh = np.linspace(b.min(), b.max(), NTANH) * 0.97

    G = _fit_G(a.ravel(), b.ravel(), ascale, bsh)

    # Mixing stationaries.  Wqfd[q, f, d] couples Chebyshev q with
    # B-feature f for hid2 channel d.  The linear feature (f=1) carries
    # RAW tm on the B side, so its hid2_bias part is folded into the
    # constant feature column; out_bias folded into (q=0, f=0, d=0).
    Wqfd = np.einsum('qf,d->qfd', G, w)
    Wqfd[:, 0, :] += np.outer(G[:, 1], w * h2b)
    Wqfd[0, 0, 0] += float(out_bias)

    # diagonal mask for the on-device smix build (bf16-exact 0/1)
    dd = np.arange(D)
    mask = np.zeros((H, D), dtype=np.float32)
    mask[0 * D + dd, dd] = 1.0
    mask[1 * D + dd, dd] = 1.0
    mask = tobf(mask)

    # biases image: [cbm, cbh, tanh-pair biases, -, -, 1/ascale, h2b, -,
    #                smix value vectors (bf16-rounded so mask*val is exact)]
    biases = np.zeros((H, 8 + 2 * NS * NCH), dtype=np.float32)
    for s in range(NS):
        for c in range(NCH):
            for fl in range(2):
                vcol = 8 + 2 * (s * NCH + c) + fl
                for ql in range(2):
                    biases[ql * D + dd, vcol] = Wqfd[2 * s + ql,
                                                     2 * c + fl, dd]
    biases[:, 8:] = bfval(biases[:, 8:])
    biases[:, 0] = cat_bias[H:]
    biases[:, 1] = cat_bias[:H]
    for k in range(NTP):
        for fl in range(2):
            biases[fl * D + dd, 2 + k] = bsh[2 * k + fl] + h2b[dd]
    biases[:, 5] = 1.0 / ascale
    biases[0 * D + dd, 6] = h2b[dd]
    biases[1 * D + dd, 6] = h2b[dd]

    in_maps = []
    for c in range(NCORES):
        xtmc = np.ascontiguousarray(xt[:, c * R:(c + 1) * R])
        m = {"xtmp": tobf(pack(xtmc)), "aw": aw, "wfomp": wfomp,
             "mask": mask, "bias": biases}
        for ck in range(NCHK):
            m[f"xtq{ck}"] = xtq[ck]
        in_maps.append(m)
    return in_maps


def kernel(x, W_foh, W_fom, cat_bias, W_hid2, hid2_bias, W_out, out_bias):
    x = np.asarray(x, dtype=np.float32)
    W_foh = np.asarray(W_foh, dtype=np.float32)
    W_fom = np.asarray(W_fom, dtype=np.float32)
    cat_bias = np.asarray(cat_bias, dtype=np.float32)
    W_hid2 = np.asarray(W_hid2, dtype=np.float32)
    hid2_bias = np.asarray(hid2_bias, dtype=np.float32)
    W_out = np.asarray(W_out, dtype=np.float32)
    out_bias = np.asarray(out_bias, dtype=np.float32)

    nc = _build_program()
    in_maps = _make_in_maps(x, W_foh, W_fom, cat_bias, W_hid2, hid2_bias,
                            W_out, float(out_bias[0]))
    res = run_bass_kernel_spmd(nc, in_maps, list(range(NCORES)))
    out = np.concatenate([res.results[c]["out"] for c in range(NCORES)],
                         axis=0)
    return out.astype(np.float32)


if __name__ == "__main__":
    rng = np.random.default_rng(0)
    ins = {
        "x": rng.standard_normal((N, 2, F // 2), dtype=np.float32),
        "W_foh": rng.standard_normal((F, H), dtype=np.float32) * 0.05,
        "W_fom": rng.standard_normal((F, H), dtype=np.float32) * 0.05,
        "cat_bias": rng.standard_normal((2 * H,), dtype=np.float32) * 0.05,
        "W_hid2": rng.standard_normal((2 * H, D), dtype=np.float32) * 0.05,
        "hid2_bias": rng.standard_normal((D,), dtype=np.float32) * 0.05,
        "W_out": rng.standard_normal((D, 1), dtype=np.float32) * 0.05,
        "out_bias": rng.standard_normal((1,), dtype=np.float32) * 0.05,
    }
    out = kernel(**ins)
    print("out", out.shape, out.dtype, out[:2, :4])


# revision 14
# speedup vs baseline: 2.0865x; 2.0865x over previous
"""Trainium2 Bass kernel for nn_ConcatHeadModule (pairwise MLP scores).

scores[i, j] = W_out . tanh(th[i] + tm[j] + hid2_bias) + out_bias
  th = tanh(xf @ W_foh + cat_bias[:H]) @ W_hid2[:H]
  tm = tanh(xf @ W_fom + cat_bias[H:]) @ W_hid2[H:]

tanh(a+b) is replaced by a low-rank separable expansion fitted on the
empirical (a, b) value distribution (host-side LS fit on quantile
grids):

  tanh(a+b) ~= sum_{q<Q} sum_{f<NB} T_q(a/ascale) * G[q,f] * g_f(b)

with T_q Chebyshev polynomials (stride-2 DVE recurrence, 128 wide) and
g_f in {1, b, tanh(b + s_k)} (+ optionally {b^2, b^3}).  The pairwise
scores then become 4 accumulating bf16 matmuls per output column chunk
with contraction dim NB*D = 512.

Layout/schedule (v2):
 - B side is COLUMN-CHUNKED: x^T arrives as 4 column-quarter images on
   3 DMA queues; each quarter flows tm -> tanhm -> ptm(half) ->
   B-feature ACTs(half) -> final matmuls(quarter) -> evac -> store,
   so output DMA overlaps compute.
 - th/tm projections use duplicated stationaries [w|w] so PSUM holds
   results on BOTH partition halves; feature ACTs run full-width
   straight from PSUM.
 - Mixing runs s-outer in two passes over two PSUM banks so it
   pipelines behind the Chebyshev recurrence.
 - Final contraction is bf16 (At/Bt bf16), PSUM f32.
 - All engines are used: PE (matmuls), Scalar (tanh ACTs), DVE
   (recurrence + casts + evac), GpSimd (copies + evac + DMA), Sync/
   Scalar/GpSimd DMA queues balanced by arrival deadlines.

Sharding: rows i split across 8 cores (128 rows each); everything else
replicated.
"""

import sys

sys.path.insert(0, "/opt/trn_rl_repo")

import numpy as np

import concourse.bass as bass
from concourse.alu_op_type import AluOpType
import concourse.tile as tile
from concourse import bacc, mybir
from concourse.bass_utils import run_bass_kernel_spmd

N = 1024          # nodes
F = 512           # 2 * LDIMS
H = 128           # hidden
D = 64            # hid2
NCORES = 8
R = N // NCORES   # rows per core = 128

Q = 8             # Chebyshev degree count (a-side)
NS = Q // 2       # Chebyshev pair tiles = 4
NB = 8            # b-side features
NCH = NB // 2     # 128-partition chunks in the final contraction = 4
NPOLY = 0         # 0 (safe: 6 tanh feats) or 2 (mixed: b^2,b^3 + 4 tanh)
NTANH = NB - 2 - NPOLY
NTP = NTANH // 2  # tanh feature pairs on scalar engine

CLEN = 256        # output column chunk
NCHK = N // CLEN  # 4
HLEN = 512        # half width (ptm / B-ACT granularity)

F32 = mybir.dt.float32
BF16 = mybir.dt.bfloat16
Tanh = mybir.ActivationFunctionType.Tanh
Copy = mybir.ActivationFunctionType.Copy


def _cheb(x, n):
    T = np.empty(x.shape + (n,))
    T[..., 0] = 1.0
    T[..., 1] = x
    for q in range(2, n):
        T[..., q] = 2 * x * T[..., q - 1] - T[..., q - 2]
    return T


def _build_program(out_bias: float = 0.0):
    # out_bias is folded into the smix input data; the program itself is
    # independent of it (arg kept for test-harness compatibility).
    nc = bacc.Bacc("TRN2", target_bir_lowering=False, debug=False,
                   num_devices=NCORES)

    # DRAM inputs.  aw = [wfohp (4H) | wh2dup (4D)] on the gpsimd queue.
    # smix is built ON DEVICE (it is 1.6% dense block-diagonal): a [H, D]
    # 0/1 diagonal mask is scaled per-partition by value vectors shipped
    # in the bias image (cols 8..8+2*NS*NCH).
    xtq_d = [nc.dram_tensor(f"xtq{c}", [H, 4 * CLEN], BF16,
                            kind="ExternalInput") for c in range(NCHK)]
    xtmp_d = nc.dram_tensor("xtmp", [H, 4 * R], BF16, kind="ExternalInput")
    aw_d = nc.dram_tensor("aw", [H, 4 * H + 4 * D], BF16,
                          kind="ExternalInput")
    wfomp_d = nc.dram_tensor("wfomp", [H, 4 * H], BF16, kind="ExternalInput")
    mask_d = nc.dram_tensor("mask", [H, D], BF16, kind="ExternalInput")
    bias_d = nc.dram_tensor("bias", [H, 8 + 2 * NS * NCH], F32,
                            kind="ExternalInput")
    out_d = nc.dram_tensor("out", [R, N], F32, kind="ExternalOutput")

    with tile.TileContext(nc) as tc:
        with (
            tc.tile_pool(name="consts", bufs=1) as consts,
            tc.tile_pool(name="work", bufs=1) as work,
            tc.tile_pool(name="scr", bufs=2) as scrp,
            tc.tile_pool(name="stage", bufs=1) as stagep,
            tc.tile_pool(name="ps", bufs=2, space="PSUM") as psp,
            tc.tile_pool(name="ptm", bufs=2, space="PSUM") as ptmp,
            tc.tile_pool(name="ptile", bufs=2, space="PSUM") as tmp_,
            tc.tile_pool(name="pmix", bufs=2, space="PSUM") as pmixp,
        ):
            # Trigger the tanh ACT table load immediately.
            warm = consts.tile([H, 1], F32, tag="warm")
            nc.vector.memset(warm[:], 0.0)

            # ---- input DMAs, balanced by arrival deadline ----
            xtq = [consts.tile([H, 4 * CLEN], BF16, tag=f"xtq{c}",
                               name=f"xtq{c}") for c in range(NCHK)]
            xtmp = consts.tile([H, 4 * R], BF16, tag="xtmp")
            aw = consts.tile([H, 4 * H + 4 * D], BF16, tag="aw")
            wfomp = consts.tile([H, 4 * H], BF16, tag="wfomp")
            mask = consts.tile([H, D], BF16, tag="mask")
            smix = work.tile([H, NS * NCH * H], BF16, tag="smix")
            biases = consts.tile([H, 8 + 2 * NS * NCH], F32, tag="biases")

            # sync queue: xtmp, bias, xtq1, xtq3 (+ outputs 0,1 later)
            nc.sync.dma_start(xtmp[:], xtmp_d[:])
            nc.sync.dma_start(biases[:], bias_d[:])
            nc.sync.dma_start(xtq[1][:], xtq_d[1][:])
            nc.sync.dma_start(xtq[3][:], xtq_d[3][:])
            # scalar queue: wfomp, xtq0 (engine then does ACT chain)
            nc.scalar.dma_start(wfomp[:], wfomp_d[:])
            nc.scalar.dma_start(xtq[0][:], xtq_d[0][:])
            # gpsimd queue: mask, aw, xtq2 (+ outputs 2,3 later)
            nc.gpsimd.dma_start(mask[:], mask_d[:])
            nc.gpsimd.dma_start(aw[:], aw_d[:])
            nc.gpsimd.dma_start(xtq[2][:], xtq_d[2][:])

            nc.scalar.activation(warm[:], warm[:], Tanh)

            # ---- build smix on GpSimd: 32 x [H, D] per-partition scaled
            # copies of the diagonal mask (s-major so pass A unblocks first)
            for s in range(NS):
                for c in range(NCH):
                    blk = (s * NCH + c) * H
                    for fl in range(2):
                        vcol = 8 + 2 * (s * NCH + c) + fl
                        nc.vector.tensor_scalar_mul(
                            smix[:, blk + fl * D:blk + (fl + 1) * D],
                            mask[:], biases[:, vcol:vcol + 1])

            wfohp = aw[:, 0:4 * H]
            wh2t = aw[:, 4 * H:4 * H + 2 * D]        # [wh2t|wh2t]
            wh2b = aw[:, 4 * H + 2 * D:4 * H + 4 * D]  # [wh2b|wh2b]
            cbm = biases[:, 0:1]
            cbh = biases[:, 1:2]
            rascale = biases[:, 5:6]

            # B-feature tiles (bf16), full width, written per chunk.
            Bt = [work.tile([2 * D, N], BF16, tag=f"B{c}", name=f"B{c}")
                  for c in range(NCH)]
            # ones feature: top half of Bt[0]
            nc.vector.memset(Bt[0][0:D, :], 1.0)

            # ---- A side: proj-h -> tanhh -> th ----
            pm2 = psp.tile([H, CLEN], F32, tag="ps", name="pm2")
            for q in range(4):
                nc.tensor.matmul(pm2[:, 0:R], wfohp[:, q * H:(q + 1) * H],
                                 xtmp[:, q * R:(q + 1) * R],
                                 start=(q == 0), stop=(q == 3))
            tanhh = work.tile([H, R], BF16, tag="tanhh")
            nc.scalar.activation(tanhh[:], pm2[:, 0:R], Tanh, bias=cbh)
            ps3 = psp.tile([H, CLEN], F32, tag="ps", name="ps3")
            nc.tensor.matmul(ps3[:, 0:R], wh2t, tanhh[:],
                             start=True, stop=True)

            # ---- B side per-chunk: tm -> tanhm ----
            tanhm = [work.tile([H, CLEN], BF16, tag=f"tanhm{c}",
                               name=f"tanhm{c}") for c in range(NCHK)]

            def tm_chunk(c):
                pm = tmp_.tile([H, CLEN], F32, tag="tm", name=f"tm{c}")
                for q in range(4):
                    nc.tensor.matmul(pm[:], wfomp[:, q * H:(q + 1) * H],
                                     xtq[c][:, q * CLEN:(q + 1) * CLEN],
                                     start=(q == 0), stop=(q == 3))
                nc.scalar.activation(tanhm[c][:], pm[:], Tanh, bias=cbm)

            # ---- A-side Chebyshev chain on DVE ----
            def a_chain():
                arep = work.tile([2 * D, R], F32, tag="arep")
                nc.vector.tensor_scalar_mul(arep[:], ps3[:, 0:R], rascale)
                sq = work.tile([2 * D, R], F32, tag="sq")
                nc.vector.tensor_mul(sq[:], arep[:], arep[:])
                M2 = work.tile([2 * D, R], F32, tag="M2")
                nc.vector.tensor_scalar(M2[:], sq[:], 4.0, -2.0,
                                        AluOpType.mult, AluOpType.add)
                Pf = [work.tile([2 * D, R], F32, tag=f"Pf{s}", name=f"Pf{s}")
                      for s in range(NS)]
                Pb = [work.tile([2 * D, R], BF16, tag=f"Pb{s}",
                                name=f"Pb{s}") for s in range(NS)]
                nc.vector.memset(Pf[0][0:D, :], 1.0)
                nc.vector.tensor_copy(Pf[0][D:2 * D, :], arep[D:2 * D, :])
                nc.vector.tensor_copy(Pb[0][:], Pf[0][:])
                nc.vector.tensor_scalar(Pf[1][0:D, :], sq[0:D, :], 2.0, -1.0,
                                        AluOpType.mult, AluOpType.add)
                scr0 = scrp.tile([2 * D, R], F32, tag="scr")
                nc.vector.tensor_mul(scr0[D:2 * D, :], M2[D:2 * D, :],
                                     arep[D:2 * D, :])
                nc.vector.tensor_sub(Pf[1][D:2 * D, :], scr0[D:2 * D, :],
                                     arep[D:2 * D, :])
                nc.vector.tensor_copy(Pb[1][:], Pf[1][:])
                for s in range(2, NS):
                    scr = scrp.tile([2 * D, R], F32, tag="scr2",
                                    name=f"scr{s}")
                    nc.vector.tensor_mul(scr[:], M2[:], Pf[s - 1][:])
                    nc.vector.tensor_sub(Pf[s][:], scr[:], Pf[s - 2][:])
                    nc.vector.tensor_copy(Pb[s][:], Pf[s][:])
                return Pb

            # ---- schedule ----
            tm_chunk(0)
            a_chain_Pb = a_chain()
            tm_chunk(1)

            # ptm half 0
            ptm0 = ptmp.tile([2 * D, HLEN], F32, tag="ptm", name="ptm0")
            nc.tensor.matmul(ptm0[:, 0:CLEN], wh2b, tanhm[0][:],
                             start=True, stop=True, skip_group_check=True)
            nc.tensor.matmul(ptm0[:, CLEN:], wh2b, tanhm[1][:],
                             start=True, stop=True, skip_group_check=True)

            # B features half 0 (scalar tanh pairs; gpsimd raw-b copy)
            for k in range(NTP):
                nc.scalar.activation(Bt[1 + NPOLY // 2 + k][:, 0:HLEN],
                                     ptm0[:], Tanh,
                                     bias=biases[:, 2 + k:3 + k])
            nc.vector.tensor_copy(Bt[0][D:2 * D, 0:HLEN], ptm0[D:2 * D, :])
            if NPOLY:
                h2bv = biases[:, 6:7]
                bb0 = scrp.tile([2 * D, HLEN], F32, tag="bb", name="bb0")
                nc.vector.tensor_scalar(bb0[:], ptm0[:], h2bv, 0.0,
                                        AluOpType.add, AluOpType.bypass)
                zz0 = scrp.tile([2 * D, HLEN], F32, tag="zz", name="zz0")
                nc.vector.tensor_mul(zz0[:], bb0[:], bb0[:])
                nc.vector.tensor_copy(Bt[1][0:D, 0:HLEN], zz0[0:D, :])
                nc.vector.tensor_mul(Bt[1][D:2 * D, 0:HLEN],
                                     zz0[D:2 * D, :], bb0[D:2 * D, :])

            # mixing pass A (chunks 0,1), s-outer
            pA = [pmixp.tile([H, R], F32, tag="pmix", name=f"pA{c}")
                  for c in range(NCH)]
            At = [work.tile([H, R], BF16, tag=f"A{c}", name=f"A{c}")
                  for c in range(NCH)]
            tm_chunk(2)
            for s in range(NS):
                for c in (0, 1):
                    blk = (s * NCH + c) * H
                    nc.tensor.matmul(pA[c][:], smix[:, blk:blk + H],
                                     a_chain_Pb[s][:],
                                     start=(s == 0), stop=(s == NS - 1),
                                     skip_group_check=True)
            nc.vector.tensor_copy(At[0][:], pA[0][:])
            nc.vector.tensor_copy(At[1][:], pA[1][:])

            tm_chunk(3)
            # ptm half 1
            ptm1 = ptmp.tile([2 * D, HLEN], F32, tag="ptm", name="ptm1")
            nc.tensor.matmul(ptm1[:, 0:CLEN], wh2b, tanhm[2][:],
                             start=True, stop=True, skip_group_check=True)
            nc.tensor.matmul(ptm1[:, CLEN:], wh2b, tanhm[3][:],
                             start=True, stop=True, skip_group_check=True)

            # mixing pass B (chunks 2,3)
            for s in range(NS):
                for c in (2, 3):
                    blk = (s * NCH + c) * H
                    nc.tensor.matmul(pA[c][:], smix[:, blk:blk + H],
                                     a_chain_Pb[s][:],
                                     start=(s == 0), stop=(s == NS - 1),
                                     skip_group_check=True)
            nc.vector.tensor_copy(At[2][:], pA[2][:])
            nc.vector.tensor_copy(At[3][:], pA[3][:])

            # B features half 1
            for k in range(NTP):
                nc.scalar.activation(Bt[1 + NPOLY // 2 + k][:, HLEN:],
                                     ptm1[:], Tanh,
                                     bias=biases[:, 2 + k:3 + k])
            nc.vector.tensor_copy(Bt[0][D:2 * D, HLEN:], ptm1[D:2 * D, :])
            if NPOLY:
                h2bv = biases[:, 6:7]
                bb1 = scrp.tile([2 * D, HLEN], F32, tag="bb", name="bb1")
                nc.vector.tensor_scalar(bb1[:], ptm1[:], h2bv, 0.0,
                                        AluOpType.add, AluOpType.bypass)
                zz1 = scrp.tile([2 * D, HLEN], F32, tag="zz", name="zz1")
                nc.gpsimd.tensor_mul(zz1[:], bb1[:], bb1[:])
                nc.gpsimd.tensor_copy(Bt[1][0:D, HLEN:], zz1[0:D, :])
                nc.gpsimd.tensor_mul(Bt[1][D:2 * D, HLEN:],
                                     zz1[D:2 * D, :], bb1[D:2 * D, :])

            # ---- final contraction + store, per column chunk ----
            evac_engine = [nc.vector, nc.vector, nc.scalar, nc.scalar]
            dma_engine = [nc.sync, nc.sync, nc.gpsimd, nc.gpsimd]
            for ck in range(NCHK):
                mv = slice(ck * CLEN, (ck + 1) * CLEN)
                psc = psp.tile([H, CLEN], F32, tag="ps", name=f"psc{ck}")
                corder = (1, 2, 3, 0)   # raw-b feature pair arrives last
                for oi, ci in enumerate(corder):
                    nc.tensor.matmul(psc[:], At[ci][:], Bt[ci][:, mv],
                                     start=(oi == 0), stop=(oi == NCH - 1),
                                     skip_group_check=True)
                stg = stagep.tile([R, CLEN], F32, tag=f"stg{ck}",
                                  name=f"stg{ck}")
                if evac_engine[ck] is nc.scalar:
                    nc.scalar.activation(stg[:], psc[:], Copy)
                else:
                    evac_engine[ck].tensor_copy(stg[:], psc[:])
                dma_engine[ck].dma_start(out_d[:, mv], stg[:])

    nc.compile()
    return nc


def _fit_G(a_samp, b_samp, ascale, bsh):
    """LS fit of tanh(a+b) on empirical quantile grids."""
    na = 301
    qs = np.linspace(0, 1, na)
    ag = np.quantile(a_samp, qs)
    ag = np.concatenate([ag, np.linspace(ag[0] * 1.08, ag[-1] * 1.08, 32)])
    bg = np.quantile(b_samp, qs)
    bg = np.concatenate([bg, np.linspace(bg[0] * 1.08, bg[-1] * 1.08, 32)])
    M = np.tanh(ag[:, None] + bg[None, :])
    Fa = _cheb(np.clip(ag / ascale, -1, 1), Q)
    feats = [np.ones_like(bg), bg]
    for i in range(NPOLY):
        feats.append(bg ** (2 + i))
    for c in bsh:
        feats.append(np.tanh(bg + c))
    Fb = np.stack(feats, 1)
    lam = 1e-7
    G = np.linalg.solve(Fa.T @ Fa + lam * np.eye(Q), Fa.T @ M @ Fb)
    G = G @ np.linalg.inv(Fb.T @ Fb + lam * np.eye(NB))
    return G


def _make_in_maps(x, W_foh, W_fom, cat_bias, W_hid2, hid2_bias, W_out,
                  out_bias=0.0):
    import ml_dtypes

    def tobf(a):
        return np.asarray(a, np.float32).astype(ml_dtypes.bfloat16)

    def bfval(a):
        return np.asarray(a, np.float32).astype(
            ml_dtypes.bfloat16).astype(np.float32)

    xf = x.reshape(N, F)
    xt = np.ascontiguousarray(xf.T)                      # [F, N]

    # p-major packing: img[p, q*C + j] = src[q*128 + p, j]
    def pack(src):
        C = src.shape[1]
        return np.ascontiguousarray(
            src.reshape(4, H, C).transpose(1, 0, 2).reshape(H, 4 * C))

    xtp = pack(xt)                                       # [H, 4*N]
    # column chunks: [H, 4*CLEN] each
    xtq = [tobf(np.ascontiguousarray(
        xtp.reshape(H, 4, N)[:, :, c * CLEN:(c + 1) * CLEN]
        .reshape(H, 4 * CLEN))) for c in range(NCHK)]
    wfomp = tobf(pack(W_fom))
    wfohp = pack(W_foh)
    wh2dup = np.concatenate([W_hid2[:H], W_hid2[:H],
                             W_hid2[H:], W_hid2[H:]], axis=1)
    aw = tobf(np.concatenate([wfohp, wh2dup], axis=1))

    # --- empirical a/b samples (match device numerics: bf16 inputs) ---
    w = W_out[:, 0].astype(np.float64)
    h2b = hid2_bias.astype(np.float64)
    xq = bfval(xf)
    headfov = xq @ bfval(W_foh)
    modfov = xq @ bfval(W_fom)
    tanhh = bfval(np.tanh(headfov + cat_bias[:H]))
    tanhm = bfval(np.tanh(modfov + cat_bias[H:]))
    wh2q = bfval(W_hid2)
    a = tanhh @ wh2q[:H]
    b = tanhm @ wh2q[H:] + h2b
    ascale = float(np.abs(a).max()) * 1.02
    bsh = np.linspace(b.min(), b.max(), NTANH) * 0.97

    G = _fit_G(a.ravel(), b.ravel(), ascale, bsh)

    # Mixing stationaries.  Wqfd[q, f, d] couples Chebyshev q with
    # B-feature f for hid2 channel d.  The linear feature (f=1) carries
    # RAW tm on the B side, so its hid2_bias part is folded into the
    # constant feature column; out_bias folded into (q=0, f=0, d=0).
    Wqfd = np.einsum('qf,d->qfd', G, w)
    Wqfd[:, 0, :] += np.outer(G[:, 1], w * h2b)
    Wqfd[0, 0, 0] += float(out_bias)

    # diagonal mask for the on-device smix build (bf16-exact 0/1)
    dd = np.arange(D)
    mask = np.zeros((H, D), dtype=np.float32)
    mask[0 * D + dd, dd] = 1.0
    mask[1 * D + dd, dd] = 1.0
    mask = tobf(mask)

    # biases image: [cbm, cbh, tanh-pair biases, -, -, 1/ascale, h2b, -,
    #                smix value vectors (bf16-rounded so mask*val is exact)]
    biases = np.zeros((H, 8 + 2 * NS * NCH), dtype=np.float32)
    for s in range(NS):
        for c in range(NCH):
            for fl in range(2):
                vcol = 8 + 2 * (s * NCH + c) + fl
                for ql in range(2):
                    biases[ql * D + dd, vcol] = Wqfd[2 * s + ql,
                                                     2 * c + fl, dd]
    biases[:, 8:] = bfval(biases[:, 8:])
    biases[:, 0] = cat_bias[H:]
    biases[:, 1] = cat_bias[:H]
    for k in range(NTP):
        for fl in range(2):
            biases[fl * D + dd, 2 + k] = bsh[2 * k + fl] + h2b[dd]
    biases[:, 5] = 1.0 / ascale
    biases[0 * D + dd, 6] = h2b[dd]
    biases[1 * D + dd, 6] = h2b[dd]

    in_maps = []
    for c in range(NCORES):
        xtmc = np.ascontiguousarray(xt[:, c * R:(c + 1) * R])
        m = {"xtmp": tobf(pack(xtmc)), "aw": aw, "wfomp": wfomp,
             "mask": mask, "bias": biases}
        for ck in range(NCHK):
            m[f"xtq{ck}"] = xtq[ck]
        in_maps.append(m)
    return in_maps


def kernel(x, W_foh, W_fom, cat_bias, W_hid2, hid2_bias, W_out, out_bias):
    x = np.asarray(x, dtype=np.float32)
    W_foh = np.asarray(W_foh, dtype=np.float32)
    W_fom = np.asarray(W_fom, dtype=np.float32)
    cat_bias = np.asarray(cat_bias, dtype=np.float32)
    W_hid2 = np.asarray(W_hid2, dtype=np.float32)
    hid2_bias = np.asarray(hid2_bias, dtype=np.float32)
    W_out = np.asarray(W_out, dtype=np.float32)
    out_bias = np.asarray(out_bias, dtype=np.float32)

    nc = _build_program()
    in_maps = _make_in_maps(x, W_foh, W_fom, cat_bias, W_hid2, hid2_bias,
                            W_out, float(out_bias[0]))
    res = run_bass_kernel_spmd(nc, in_maps, list(range(NCORES)))
    out = np.concatenate([res.results[c]["out"] for c in range(NCORES)],
                         axis=0)
    return out.astype(np.float32)


if __name__ == "__main__":
    rng = np.random.default_rng(0)
    ins = {
        "x": rng.standard_normal((N, 2, F // 2), dtype=np.float32),
        "W_foh": rng.standard_normal((F, H), dtype=np.float32) * 0.05,
        "W_fom": rng.standard_normal((F, H), dtype=np.float32) * 0.05,
        "cat_bias": rng.standard_normal((2 * H,), dtype=np.float32) * 0.05,
        "W_hid2": rng.standard_normal((2 * H, D), dtype=np.float32) * 0.05,
        "hid2_bias": rng.standard_normal((D,), dtype=np.float32) * 0.05,
        "W_out": rng.standard_normal((D, 1), dtype=np.float32) * 0.05,
        "out_bias": rng.standard_normal((1,), dtype=np.float32) * 0.05,
    }
    out = kernel(**ins)
    print("out", out.shape, out.dtype, out[:2, :4])


# revision 15
# speedup vs baseline: 2.3719x; 1.1368x over previous
"""Trainium2 Bass kernel for nn_ConcatHeadModule (pairwise MLP scores).

scores[i, j] = W_out . tanh(th[i] + tm[j] + hid2_bias) + out_bias
  th = tanh(xf @ W_foh + cat_bias[:H]) @ W_hid2[:H]
  tm = tanh(xf @ W_fom + cat_bias[H:]) @ W_hid2[H:]

tanh(a+b) is replaced by a low-rank separable expansion fitted on the
empirical (a, b) value distribution (host-side LS fit on quantile
grids):

  tanh(a+b) ~= sum_{q<Q} sum_{f<NB} T_q(a/ascale) * G[q,f] * g_f(b)

with T_q Chebyshev polynomials (stride-2 DVE recurrence, 128 wide) and
g_f in {1, b, tanh(b + s_k)}.  The pairwise scores then become 4
accumulating bf16 matmuls per output column chunk with contraction dim
NB*D = 512.

v4 layout notes (driven by the DMA descriptor-rate floor ~20-30ns/row:
narrow transfers cost ~128 rows x 30ns regardless of size):
 - Each core's own 128 columns are ROLLED to the front of its x^T
   image, so the A-side projection reads them from the first B chunk
   (no separate xtmp transfer); host un-rolls the output columns.
 - All weights + f32 biases (bit-packed into bf16 columns) ship as ONE
   wide `wts` image; x^T ships as [chunk01 | chunk2 | chunk3] split
   across the three queues; smix ships as one 4KB-row image.
 - B side is column-chunked: tm per quarter, tanhm/ptm/B-ACTs per
   half, final matmuls + evac per quarter, stores per half on two
   queues, so output DMA overlaps compute.
 - th/tm use duplicated stationaries [w|w] so PSUM holds results on
   BOTH partition halves; feature ACTs run full-width from PSUM.
 - Mixing runs s-outer in two passes over two PSUM banks, pipelined
   behind the Chebyshev recurrence; final contraction is bf16.

Sharding: rows i split across 8 cores (128 rows each).
"""

import sys

sys.path.insert(0, "/opt/trn_rl_repo")

import numpy as np

import concourse.bass as bass
from concourse.alu_op_type import AluOpType
import concourse.tile as tile
from concourse import bacc, mybir
from concourse.bass_utils import run_bass_kernel_spmd

N = 1024          # nodes
F = 512           # 2 * LDIMS
H = 128           # hidden
D = 64            # hid2
NCORES = 8
R = N // NCORES   # rows per core = 128

Q = 8             # Chebyshev degree count (a-side)
NS = Q // 2       # Chebyshev pair tiles = 4
NB = 8            # b-side features
NCH = NB // 2     # final-contraction chunks = 4
NTP = 3           # tanh feature pairs on scalar engine

CLEN = 256        # output column chunk
NCHK = N // CLEN  # 4
HLEN = 512        # half width

WFOH0 = 0                 # wts column layout (bf16 cols)
WH2_0 = 4 * H             # 512: [wh2t|wh2t|wh2b|wh2b]
WFOM0 = WH2_0 + 4 * D     # 768
BIAS0 = WFOM0 + 4 * H     # 1280: 8 f32 cols bit-packed as 16 bf16 cols
WTSW = BIAS0 + 16         # 1296

F32 = mybir.dt.float32
BF16 = mybir.dt.bfloat16
Tanh = mybir.ActivationFunctionType.Tanh
Copy = mybir.ActivationFunctionType.Copy


def _cheb(x, n):
    T = np.empty(x.shape + (n,))
    T[..., 0] = 1.0
    T[..., 1] = x
    for q in range(2, n):
        T[..., q] = 2 * x * T[..., q - 1] - T[..., q - 2]
    return T


def _build_program(out_bias: float = 0.0):
    nc = bacc.Bacc("TRN2", target_bir_lowering=False, debug=False,
                   num_devices=NCORES)

    xtq01_d = nc.dram_tensor("xtq01", [H, 8 * CLEN], BF16,
                             kind="ExternalInput")
    xtq2_d = nc.dram_tensor("xtq2", [H, 4 * CLEN], BF16,
                            kind="ExternalInput")
    xtq3_d = nc.dram_tensor("xtq3", [H, 4 * CLEN], BF16,
                            kind="ExternalInput")
    wts_d = nc.dram_tensor("wts", [H, WTSW], BF16, kind="ExternalInput")
    smix_d = nc.dram_tensor("smix", [H, NS * NCH * H], BF16,
                            kind="ExternalInput")
    out_d = nc.dram_tensor("out", [R, N], F32, kind="ExternalOutput")

    with tile.TileContext(nc) as tc:
        with (
            tc.tile_pool(name="consts", bufs=1) as consts,
            tc.tile_pool(name="work", bufs=1) as work,
            tc.tile_pool(name="scr", bufs=2) as scrp,
            tc.tile_pool(name="stage", bufs=1) as stagep,
            tc.tile_pool(name="ps", bufs=2, space="PSUM") as psp,
            tc.tile_pool(name="ptm", bufs=2, space="PSUM") as ptmp,
            tc.tile_pool(name="ptile", bufs=2, space="PSUM") as tmp_,
            tc.tile_pool(name="pmix", bufs=2, space="PSUM") as pmixp,
        ):
            warm = consts.tile([H, 1], F32, tag="warm")
            nc.vector.memset(warm[:], 0.0)

            xtq01 = consts.tile([H, 8 * CLEN], BF16, tag="xtq01")
            xtq2 = consts.tile([H, 4 * CLEN], BF16, tag="xtq2")
            xtq3 = consts.tile([H, 4 * CLEN], BF16, tag="xtq3")
            wts = consts.tile([H, WTSW], BF16, tag="wts")
            smix = consts.tile([H, NS * NCH * H], BF16, tag="smix")

            # sync: xtq01, xtq2 (+ out half 0); scalar: wts;
            # gpsimd: smix, xtq3 (+ out half 1)
            nc.sync.dma_start(xtq01[:], xtq01_d[:])
            nc.scalar.dma_start(wts[:], wts_d[:])
            nc.gpsimd.dma_start(smix[:], smix_d[:])
            nc.sync.dma_start(xtq2[:], xtq2_d[:])
            nc.gpsimd.dma_start(xtq3[:], xtq3_d[:])

            nc.scalar.activation(warm[:], warm[:], Tanh)

            wfohp = wts[:, WFOH0:WFOH0 + 4 * H]
            wh2t = wts[:, WH2_0:WH2_0 + 2 * D]
            wh2b = wts[:, WH2_0 + 2 * D:WH2_0 + 4 * D]
            wfomp = wts[:, WFOM0:WFOM0 + 4 * H]
            biasb = wts[:, BIAS0:BIAS0 + 16].bitcast(F32)  # [H, 8] f32
            biases = work.tile([H, 8], F32, tag="biases")
            nc.vector.tensor_copy(biases[:], biasb)
            cbm = biases[:, 0:1]
            cbh = biases[:, 1:2]
            rascale = biases[:, 5:6]

            # chunk c -> (tile, base column within tile)
            chunk_src = [(xtq01, 0), (xtq01, 4 * CLEN),
                         (xtq2, 0), (xtq3, 0)]

            Bt = [work.tile([2 * D, N], BF16, tag=f"B{c}", name=f"B{c}")
                  for c in range(NCH)]
            nc.vector.memset(Bt[0][0:D, :], 1.0)

            # ---- A side: proj-h -> tanhh -> th (own cols are chunk0's
            # first R columns of each q block) ----
            pm2 = psp.tile([H, CLEN], F32, tag="ps", name="pm2")
            for q in range(4):
                nc.tensor.matmul(pm2[:, 0:R], wfohp[:, q * H:(q + 1) * H],
                                 xtq01[:, q * CLEN:q * CLEN + R],
                                 start=(q == 0), stop=(q == 3))
            tanhh = work.tile([H, R], BF16, tag="tanhh")
            nc.scalar.activation(tanhh[:], pm2[:, 0:R], Tanh, bias=cbh)
            ps3 = psp.tile([H, CLEN], F32, tag="ps", name="ps3")
            nc.tensor.matmul(ps3[:, 0:R], wh2t, tanhh[:],
                             start=True, stop=True)

            # ---- B side: tm per chunk into a half-wide psum tile ----
            tanhm = [work.tile([H, HLEN], BF16, tag=f"tanhm{h}",
                               name=f"tanhm{h}") for h in range(2)]

            def tm_chunk(pm, c, rel):
                t, base = chunk_src[c]
                for q in range(4):
                    nc.tensor.matmul(
                        pm[:, rel * CLEN:(rel + 1) * CLEN],
                        wfomp[:, q * H:(q + 1) * H],
                        t[:, base + q * CLEN:base + (q + 1) * CLEN],
                        start=(q == 0), stop=(q == 3),
                        skip_group_check=True)

            def a_chain():
                arep = work.tile([2 * D, R], F32, tag="arep")
                nc.vector.tensor_scalar_mul(arep[:], ps3[:, 0:R], rascale)
                sq = work.tile([2 * D, R], F32, tag="sq")
                nc.vector.tensor_mul(sq[:], arep[:], arep[:])
                M2 = work.tile([2 * D, R], F32, tag="M2")
                nc.vector.tensor_scalar(M2[:], sq[:], 4.0, -2.0,
                                        AluOpType.mult, AluOpType.add)
                Pf = [work.tile([2 * D, R], F32, tag=f"Pf{s}", name=f"Pf{s}")
                      for s in range(NS)]
                Pb = [work.tile([2 * D, R], BF16, tag=f"Pb{s}",
                                name=f"Pb{s}") for s in range(NS)]
                nc.vector.memset(Pf[0][0:D, :], 1.0)
                nc.vector.tensor_copy(Pf[0][D:2 * D, :], arep[D:2 * D, :])
                nc.vector.tensor_copy(Pb[0][:], Pf[0][:])
                nc.vector.tensor_scalar(Pf[1][0:D, :], sq[0:D, :], 2.0, -1.0,
                                        AluOpType.mult, AluOpType.add)
                scr0 = scrp.tile([2 * D, R], F32, tag="scr")
                nc.vector.tensor_mul(scr0[D:2 * D, :], M2[D:2 * D, :],
                                     arep[D:2 * D, :])
                nc.vector.tensor_sub(Pf[1][D:2 * D, :], scr0[D:2 * D, :],
                                     arep[D:2 * D, :])
                nc.vector.tensor_copy(Pb[1][:], Pf[1][:])
                for s in range(2, NS):
                    scr = scrp.tile([2 * D, R], F32, tag="scr2",
                                    name=f"scr{s}")
                    nc.vector.tensor_mul(scr[:], M2[:], Pf[s - 1][:])
                    nc.vector.tensor_sub(Pf[s][:], scr[:], Pf[s - 2][:])
                    nc.vector.tensor_copy(Pb[s][:], Pf[s][:])
                return Pb

            # half 0: chunks 0,1
            pmh0 = tmp_.tile([H, HLEN], F32, tag="tm", name="pmh0")
            tm_chunk(pmh0, 0, 0)
            Pb = a_chain()
            tm_chunk(pmh0, 1, 1)
            nc.scalar.activation(tanhm[0][:], pmh0[:], Tanh, bias=cbm)
            ptm0 = ptmp.tile([2 * D, HLEN], F32, tag="ptm", name="ptm0")
            nc.tensor.matmul(ptm0[:], wh2b, tanhm[0][:],
                             start=True, stop=True)
            for k in range(NTP):
                nc.scalar.activation(Bt[1 + k][:, 0:HLEN], ptm0[:], Tanh,
                                     bias=biases[:, 2 + k:3 + k])
            nc.vector.tensor_copy(Bt[0][D:2 * D, 0:HLEN], ptm0[D:2 * D, :])

            # mixing pass A (chunks 0,1), s-outer
            pA = [pmixp.tile([H, R], F32, tag="pmix", name=f"pA{c}")
                  for c in range(NCH)]
            At = [work.tile([H, R], BF16, tag=f"A{c}", name=f"A{c}")
                  for c in range(NCH)]
            pmh1 = tmp_.tile([H, HLEN], F32, tag="tm", name="pmh1")
            tm_chunk(pmh1, 2, 0)
            for s in range(NS):
                for c in (0, 1):
                    blk = (s * NCH + c) * H
                    nc.tensor.matmul(pA[c][:], smix[:, blk:blk + H],
                                     Pb[s][:],
                                     start=(s == 0), stop=(s == NS - 1),
                                     skip_group_check=True)
            nc.vector.tensor_copy(At[0][:], pA[0][:])
            nc.vector.tensor_copy(At[1][:], pA[1][:])

            tm_chunk(pmh1, 3, 1)
            nc.scalar.activation(tanhm[1][:], pmh1[:], Tanh, bias=cbm)
            ptm1 = ptmp.tile([2 * D, HLEN], F32, tag="ptm", name="ptm1")
            nc.tensor.matmul(ptm1[:], wh2b, tanhm[1][:],
                             start=True, stop=True)

            # mixing pass B (chunks 2,3)
            for s in range(NS):
                for c in (2, 3):
                    blk = (s * NCH + c) * H
                    nc.tensor.matmul(pA[c][:], smix[:, blk:blk + H],
                                     Pb[s][:],
                                     start=(s == 0), stop=(s == NS - 1),
                                     skip_group_check=True)
            nc.vector.tensor_copy(At[2][:], pA[2][:])
            nc.vector.tensor_copy(At[3][:], pA[3][:])

            # B features half 1
            for k in range(NTP):
                nc.scalar.activation(Bt[1 + k][:, HLEN:], ptm1[:], Tanh,
                                     bias=biases[:, 2 + k:3 + k])
            nc.vector.tensor_copy(Bt[0][D:2 * D, HLEN:], ptm1[D:2 * D, :])

            # ---- final contraction per quarter; stores per half ----
            stg = [stagep.tile([R, HLEN], F32, tag=f"stg{h}",
                               name=f"stg{h}") for h in range(2)]
            evac_engine = [nc.vector, nc.vector, nc.scalar, nc.scalar]
            corder = (1, 2, 3, 0)   # raw-b feature pair arrives last
            for ck in range(NCHK):
                mv = slice(ck * CLEN, (ck + 1) * CLEN)
                psc = psp.tile([H, CLEN], F32, tag="ps", name=f"psc{ck}")
                for oi, ci in enumerate(corder):
                    nc.tensor.matmul(psc[:], At[ci][:], Bt[ci][:, mv],
                                     start=(oi == 0), stop=(oi == NCH - 1),
                                     skip_group_check=True)
                dst = stg[ck // 2][:, (ck % 2) * CLEN:(ck % 2 + 1) * CLEN]
                if evac_engine[ck] is nc.scalar:
                    nc.scalar.activation(dst, psc[:], Copy)
                else:
                    evac_engine[ck].tensor_copy(dst, psc[:])
                if ck == 1:
                    nc.sync.dma_start(out_d[:, 0:HLEN], stg[0][:])
                elif ck == 3:
                    nc.gpsimd.dma_start(out_d[:, HLEN:], stg[1][:])

    nc.compile()
    return nc


def _fit_G(a_samp, b_samp, ascale, bsh):
    """LS fit of tanh(a+b) on empirical quantile grids."""
    na = 301
    qs = np.linspace(0, 1, na)
    ag = np.quantile(a_samp, qs)
    ag = np.concatenate([ag, np.linspace(ag[0] * 1.08, ag[-1] * 1.08, 32)])
    bg = np.quantile(b_samp, qs)
    bg = np.concatenate([bg, np.linspace(bg[0] * 1.08, bg[-1] * 1.08, 32)])
    M = np.tanh(ag[:, None] + bg[None, :])
    Fa = _cheb(np.clip(ag / ascale, -1, 1), Q)
    feats = [np.ones_like(bg), bg] + [np.tanh(bg + c) for c in bsh]
    Fb = np.stack(feats, 1)
    lam = 1e-7
    G = np.linalg.solve(Fa.T @ Fa + lam * np.eye(Q), Fa.T @ M @ Fb)
    G = G @ np.linalg.inv(Fb.T @ Fb + lam * np.eye(NB))
    return G


def _make_in_maps(x, W_foh, W_fom, cat_bias, W_hid2, hid2_bias, W_out,
                  out_bias=0.0):
    import ml_dtypes

    def tobf(a):
        return np.asarray(a, np.float32).astype(ml_dtypes.bfloat16)

    def bfval(a):
        return np.asarray(a, np.float32).astype(
            ml_dtypes.bfloat16).astype(np.float32)

    xf = x.reshape(N, F)
    xt = np.ascontiguousarray(xf.T)                      # [F, N]

    # p-major packing: img[p, q*C + j] = src[q*128 + p, j]
    def pack(src):
        C = src.shape[1]
        return np.ascontiguousarray(
            src.reshape(4, H, C).transpose(1, 0, 2).reshape(H, 4 * C))

    wfohp = pack(W_foh)
    wfomp = pack(W_fom)
    wh2dup = np.concatenate([W_hid2[:H], W_hid2[:H],
                             W_hid2[H:], W_hid2[H:]], axis=1)

    # --- empirical a/b samples (match device numerics: bf16 inputs) ---
    w = W_out[:, 0].astype(np.float64)
    h2b = hid2_bias.astype(np.float64)
    xq = bfval(xf)
    headfov = xq @ bfval(W_foh)
    modfov = xq @ bfval(W_fom)
    tanhh = bfval(np.tanh(headfov + cat_bias[:H]))
    tanhm = bfval(np.tanh(modfov + cat_bias[H:]))
    wh2q = bfval(W_hid2)
    a = tanhh @ wh2q[:H]
    b = tanhm @ wh2q[H:] + h2b
    ascale = float(np.abs(a).max()) * 1.02
    bsh = np.linspace(b.min(), b.max(), 2 * NTP) * 0.97

    G = _fit_G(a.ravel(), b.ravel(), ascale, bsh)

    # Mixing values: Wqfd[q, f, d] = G[q,f] * w[d] (+ folds: the linear
    # feature carries RAW tm on the B side, its h2b part goes to the
    # constant column; out_bias into (0,0,0)).
    Wqfd = np.einsum('qf,d->qfd', G, w)
    Wqfd[:, 0, :] += np.outer(G[:, 1], w * h2b)
    Wqfd[0, 0, 0] += float(out_bias)
    smix = np.zeros((H, NS * NCH * H), dtype=np.float32)
    dd = np.arange(D)
    for s in range(NS):
        for c in range(NCH):
            t = np.zeros((H, H), dtype=np.float32)
            for ql in range(2):
                for fl in range(2):
                    t[ql * D + dd, fl * D + dd] = Wqfd[2 * s + ql,
                                                       2 * c + fl, dd]
            smix[:, (s * NCH + c) * H:(s * NCH + c + 1) * H] = t
    smix = tobf(smix)

    # biases image (f32): [cbm, cbh, bb1..3, 1/ascale, 0, 0]
    biases = np.zeros((H, 8), dtype=np.float32)
    biases[:, 0] = cat_bias[H:]
    biases[:, 1] = cat_bias[:H]
    for k in range(NTP):
        for fl in range(2):
            biases[fl * D + dd, 2 + k] = bsh[2 * k + fl] + h2b[dd]
    biases[:, 5] = 1.0 / ascale
    biasbits = np.ascontiguousarray(biases).view(ml_dtypes.bfloat16)

    wts = np.zeros((H, WTSW), dtype=ml_dtypes.bfloat16)
    wts[:, WFOH0:WFOH0 + 4 * H] = tobf(wfohp)
    wts[:, WH2_0:WH2_0 + 4 * D] = tobf(wh2dup)
    wts[:, WFOM0:WFOM0 + 4 * H] = tobf(wfomp)
    wts[:, BIAS0:BIAS0 + 16] = biasbits

    in_maps = []
    for c in range(NCORES):
        # roll this core's own columns to the front
        xtr = np.concatenate([xt[:, c * R:], xt[:, :c * R]], axis=1)
        xtp = tobf(pack(xtr))                 # [H, 4*N], q-major
        xtp4 = np.asarray(xtp).reshape(H, 4, N)
        def chunk(c0, c1):
            return np.ascontiguousarray(
                xtp4[:, :, c0 * CLEN:c1 * CLEN].reshape(H, -1))
        # chunk layout inside each transfer: q-major per chunk
        ch = [np.ascontiguousarray(
            xtp4[:, :, k * CLEN:(k + 1) * CLEN].reshape(H, 4 * CLEN))
            for k in range(NCHK)]
        m = {"xtq01": np.ascontiguousarray(
                 np.concatenate([ch[0], ch[1]], axis=1)),
             "xtq2": ch[2], "xtq3": ch[3],
             "wts": wts, "smix": smix}
        in_maps.append(m)
    return in_maps


def kernel(x, W_foh, W_fom, cat_bias, W_hid2, hid2_bias, W_out, out_bias):
    x = np.asarray(x, dtype=np.float32)
    W_foh = np.asarray(W_foh, dtype=np.float32)
    W_fom = np.asarray(W_fom, dtype=np.float32)
    cat_bias = np.asarray(cat_bias, dtype=np.float32)
    W_hid2 = np.asarray(W_hid2, dtype=np.float32)
    hid2_bias = np.asarray(hid2_bias, dtype=np.float32)
    W_out = np.asarray(W_out, dtype=np.float32)
    out_bias = np.asarray(out_bias, dtype=np.float32)

    nc = _build_program()
    in_maps = _make_in_maps(x, W_foh, W_fom, cat_bias, W_hid2, hid2_bias,
                            W_out, float(out_bias[0]))
    res = run_bass_kernel_spmd(nc, in_maps, list(range(NCORES)))
    # un-roll the per-core column rotation
    out = np.concatenate(
        [np.roll(res.results[c]["out"], c * R, axis=1)
         for c in range(NCORES)], axis=0)
    return out.astype(np.float32)


if __name__ == "__main__":
    rng = np.random.default_rng(0)
    ins = {
        "x": rng.standard_normal((N, 2, F // 2), dtype=np.float32),
        "W_foh": rng.standard_normal((F, H), dtype=np.float32) * 0.05,
        "W_fom": rng.standard_normal((F, H), dtype=np.float32) * 0.05,
        "cat_bias": rng.standard_normal((2 * H,), dtype=np.float32) * 0.05,
        "W_hid2": rng.standard_normal((2 * H, D), dtype=np.float32) * 0.05,
        "hid2_bias": rng.standard_normal((D,), dtype=np.float32) * 0.05,
        "W_out": rng.standard_normal((D, 1), dtype=np.float32) * 0.05,
        "out_bias": rng.standard_normal((1,), dtype=np.float32) * 0.05,
    }
    out = kernel(**ins)
    print("out", out.shape, out.dtype, out[:2, :4])
